# revision 4
# baseline (speedup 1.0000x reference)
"""Causal depthwise temporal conv (K=4) on 8 TRN2 NeuronCores.

Reference semantics (for x: [B, T, D], w: [K, D], b: [D]):
    out[bt, t, d] = sum_{j=0}^{K-1} x_pad[bt, t + j, d] * w[j, d] + b[d]
where x_pad is x left-padded with K-1 zeros along time.

Strategy (v3, fp16 + PE accumulation):
  - Tensor-parallel over channels: core m owns channels [m*512, (m+1)*512).
  - All HBM traffic in fp16 (harness gate is rel_err < 2e-2; fp16 keeps
    max-rel error ~1e-3): halves DMA bytes vs f32 -> ~94us/core roofline.
  - Host pre-transposes each core's shard to channel-major [D_sh, B, T+K-1]
    (left zero-padded, fp16). Channels sit on SBUF partitions.
  - The 3 combining adds are the expensive part on DVE (tensor_tensor is
    2x-mode at best; scalar_tensor_tensor is 1x-only, measured). So taps
    {0,1,3} run on the tensor engine as diagonal-matrix matmuls that
    accumulate FOR FREE in PSUM (diag(w_j).T @ x_shift_j). ACT evacuates
    PSUM -> SBUF fp16 fused with the bias add (ACT is 1x but alignment- and
    dtype-insensitive). DVE only does tap 2 (tensor_scalar, 4x mode) and
    one tensor_tensor combine (2x mode).
  - PSUM ping-pong: each (blk, batch) chain is split into two 2048-col
    halves; each half uses 4 PSUM banks, bufs=2 covers all 8 banks.
"""

import numpy as np

import concourse.bacc as bacc
import concourse.mybir as mybir
from concourse.tile import TileContext
from concourse import bass_utils

B = 4            # batch
T = 4096         # sequence length
D = 4096         # channels (width)
K = 4            # temporal taps
N_CORES = 8
D_SH = D // N_CORES          # 512 channels per core
P = 128                      # SBUF partitions
N_BLK = D_SH // P            # 4 channel blocks per core
TP = T + K - 1               # padded time length
W_STRIDE = K + 1             # per-blk slot in the f32 wb tile

PE_TAPS = (0, 1, 3)          # taps done as diag-matmuls into PSUM
DVE_TAPS = (2,)              # taps done on DVE (must be even shifts: 4B align)
HALF = 2048                  # psum half-chain width (4 banks)
NB = 512                     # matmul moving-block width (1 psum bank)


def _build(b=B, t=T, n_blk=N_BLK):
    nc = bacc.Bacc("TRN2")
    tp = t + K - 1
    f16 = mybir.dt.float16
    f32 = mybir.dt.float32
    npe = len(PE_TAPS)
    x = nc.dram_tensor("x", [n_blk, P, b, tp], f16, kind="ExternalInput")
    wd = nc.dram_tensor("wd", [P, n_blk * npe * P], f16, kind="ExternalInput")
    wb = nc.dram_tensor("wb", [P, n_blk * W_STRIDE], f32, kind="ExternalInput")
    out = nc.dram_tensor("out", [n_blk, P, b, t], f16, kind="ExternalOutput")
    mult, add = mybir.AluOpType.mult, mybir.AluOpType.add
    ident = mybir.ActivationFunctionType.Identity
    nhalf = t // HALF

    with TileContext(nc) as tc:
        with tc.tile_pool(name="px", bufs=4) as px, \
             tc.tile_pool(name="ps", bufs=4) as ps, \
             tc.tile_pool(name="po", bufs=4) as po, \
             tc.tile_pool(name="pw", bufs=1) as pw, \
             tc.tile_pool(name="pp", bufs=2, space="PSUM") as pp:
            wdt = pw.tile([P, n_blk * npe * P], f16, tag="wd")
            nc.sync.dma_start(wdt[:], wd[:, :])
            wt = pw.tile([P, n_blk * W_STRIDE], f32, tag="wb")
            nc.sync.dma_start(wt[:], wb[:, :])

            hp = HALF + K - 1   # per-half x tile width (covers all tap shifts)
            for blk in range(n_blk):
                def w(j, blk=blk):
                    return wt[:, blk * W_STRIDE + j:blk * W_STRIDE + j + 1]

                def wdiag(ti, blk=blk):
                    o = (blk * npe + ti) * P
                    return wdt[:, o:o + P]

                for bb in range(b):
                    for h in range(nhalf):
                        base = h * HALF
                        # Per-half loads: shorter ramp, and the sync HWDGE
                        # ring carries only loads (stores go on scalar's).
                        X = px.tile([P, hp], f16, tag="x")
                        nc.sync.dma_start(X[:], x[blk, :, bb, base:base + hp])
                        pt = pp.tile([P, HALF], f32, tag="ps")
                        # PE: psum[c] += sum_{j in PE_TAPS} diag(w_j) @ x_j
                        # Tap-major so consecutive matmuls share a stationary.
                        for ti in range(npe):
                            for c in range(HALF // NB):
                                lo = c * NB + PE_TAPS[ti]
                                nc.tensor.matmul(
                                    pt[:, c * NB:(c + 1) * NB],
                                    wdiag(ti),
                                    X[:, lo:lo + NB],
                                    start=(ti == 0),
                                    stop=(ti == npe - 1),
                                    skip_group_check=True,
                                )
                        # ACT: evacuate PSUM -> fp16, fused bias add.
                        s = ps.tile([P, HALF], f16, tag="s")
                        nc.scalar.activation(s[:], pt[:, :], ident,
                                             bias=w(K), scale=1.0)
                        # DVE: remaining even taps + one combine (2x / 4x modes).
                        y = ps.tile([P, HALF], f16, tag="y")
                        j0 = DVE_TAPS[0]
                        nc.vector.tensor_scalar_mul(
                            y[:], X[:, j0:j0 + HALF], w(j0))
                        for j in DVE_TAPS[1:]:
                            y2 = ps.tile([P, HALF], f16, tag="y")
                            nc.vector.scalar_tensor_tensor(
                                y2[:], X[:, j:j + HALF], w(j),
                                y[:], mult, add)
                            y = y2
                        o = po.tile([P, HALF], f16, tag="o")
                        nc.vector.tensor_tensor(o[:], y[:], s[:], add)
                        nc.scalar.dma_start(out[blk, :, bb, base:base + HALF],
                                            o[:])
    nc.compile()
    return nc


def _prepare(x, w, b):
    x = np.asarray(x, dtype=np.float32)
    w = np.asarray(w, dtype=np.float32)
    b = np.asarray(b, dtype=np.float32)
    npe = len(PE_TAPS)
    # channel-major, left zero-padded time, fp16: [D, B, TP]
    xp = np.zeros((D, B, TP), dtype=np.float16)
    xp[:, :, K - 1:] = x.transpose(2, 0, 1)
    wbt = np.concatenate([w.T, b[:, None]], axis=1).astype(np.float32)  # [D, 5]
    in_maps = []
    for m in range(N_CORES):
        sl = slice(m * D_SH, (m + 1) * D_SH)
        wbm = wbt[sl].reshape(N_BLK, P, W_STRIDE).transpose(1, 0, 2)
        # diag stationary matrices for the PE taps: [P, n_blk*npe*P]
        wdm = np.zeros((P, N_BLK, npe, P), dtype=np.float16)
        rng = np.arange(P)
        for blk in range(N_BLK):
            for ti, tap in enumerate(PE_TAPS):
                wdm[rng, blk, ti, rng] = w[tap, m * D_SH + blk * P + rng]
        in_maps.append({
            "x": np.ascontiguousarray(xp[sl]).reshape(N_BLK, P, B, TP),
            "wd": np.ascontiguousarray(wdm).reshape(P, N_BLK * npe * P),
            "wb": np.ascontiguousarray(wbm).reshape(P, N_BLK * W_STRIDE),
        })
    return in_maps


def _collect(results):
    out = np.empty((B, T, D), dtype=np.float32)
    for m in range(N_CORES):
        o = np.asarray(results[m]["out"]).astype(np.float32)
        o = o.reshape(D_SH, B, T)
        out[:, :, m * D_SH:(m + 1) * D_SH] = o.transpose(1, 2, 0)
    return out


def _run(in_maps, trace=False, **kwargs):
    nc = _build()
    return bass_utils.run_bass_kernel_spmd(
        nc, in_maps, core_ids=list(range(N_CORES)), trace=trace, **kwargs)


def kernel(x, w, b):
    in_maps = _prepare(x, w, b)
    try:
        res = _run(in_maps)
    except Exception:
        # Transient NRT device errors have been observed on a cold first
        # execute; one retry (fresh compile dir) clears them.
        res = _run(in_maps)
    return _collect(res.results)


# revision 5
# speedup vs baseline: 1.1619x; 1.1619x over previous
"""Causal depthwise temporal conv (K=4) on 8 TRN2 NeuronCores.

Reference semantics (for x: [B, T, D], w: [K, D], b: [D]):
    out[bt, t, d] = sum_{j=0}^{K-1} x_pad[bt, t + j, d] * w[j, d] + b[d]
where x_pad is x left-padded with K-1 zeros along time.

Strategy (v4, fp16 + PE accumulation + 3-queue DMA):
  - Tensor-parallel over channels: core m owns channels [m*512, (m+1)*512).
  - All HBM traffic in fp16 (harness gate is rel_err < 2e-2; fp16 keeps
    max-rel error ~1e-3): halves DMA bytes vs f32.
  - Taps {0,1,3} run on the tensor engine as diagonal-matrix matmuls
    accumulating for free in PSUM (adds on DVE are the scarce resource:
    tensor_tensor is 2x-mode at best, scalar_tensor_tensor is 1x-only).
    ACT evacuates PSUM -> SBUF fp16 fused with the bias add. DVE does only
    tap 2 (tensor_scalar, 4x mode) and one tensor_tensor combine (2x).
  - PSUM ping-pong: each (blk, batch) chain is split into two 2048-col
    halves; each half uses 4 PSUM banks, bufs=2 covers all 8 banks.
  - DMA: loads on the sync HWDGE ring as exact-4096B rows (a 2051-elem row
    is 4102B -> per-row runt packet, measured 15% slower) plus a tiny tail
    transfer; stores on the gpsimd SWDGE queue; weights on the scalar ring.
    Three queues overlap; the ACT engine only runs ACTIVATEs.
"""

import numpy as np

import concourse.bacc as bacc
import concourse.mybir as mybir
from concourse.tile import TileContext
from concourse import bass_utils

B = 4            # batch
T = 4096         # sequence length
D = 4096         # channels (width)
K = 4            # temporal taps
N_CORES = 8
D_SH = D // N_CORES          # 512 channels per core
P = 128                      # SBUF partitions
N_BLK = D_SH // P            # 4 channel blocks per core
TPP = 4104                   # padded time length in DRAM (4096 + 8)
W_STRIDE = K + 1             # per-blk slot in the f32 wb tile

PE_TAPS = (0, 1, 3)          # taps done as diag-matmuls into PSUM
DVE_TAPS = (2,)              # taps done on DVE (must be even shifts: 4B align)
HALF = 2048                  # psum half-chain width (4 banks)
NB = 512                     # matmul moving-block width (1 psum bank)
TAIL = 8                     # extra columns fetched for cross-half tap reads


def _build(b=B, t=T, n_blk=N_BLK):
    nc = bacc.Bacc("TRN2")
    f16 = mybir.dt.float16
    f32 = mybir.dt.float32
    npe = len(PE_TAPS)
    x = nc.dram_tensor("x", [n_blk, b, P, TPP], f16, kind="ExternalInput")
    wd = nc.dram_tensor("wd", [P, n_blk * npe * P], f16, kind="ExternalInput")
    wb = nc.dram_tensor("wb", [P, n_blk * W_STRIDE], f32, kind="ExternalInput")
    out = nc.dram_tensor("out", [n_blk, b, P, t], f16, kind="ExternalOutput")
    mult, add = mybir.AluOpType.mult, mybir.AluOpType.add
    ident = mybir.ActivationFunctionType.Identity
    nhalf = t // HALF
    hp = HALF + TAIL

    with TileContext(nc) as tc:
        with tc.tile_pool(name="px", bufs=6) as px, \
             tc.tile_pool(name="ps", bufs=4) as ps, \
             tc.tile_pool(name="po", bufs=4) as po, \
             tc.tile_pool(name="pw", bufs=1) as pw, \
             tc.tile_pool(name="pp", bufs=2, space="PSUM") as pp:
            wdt = pw.tile([P, n_blk * npe * P], f16, tag="wd")
            nc.scalar.dma_start(wdt[:], wd[:, :])
            wt = pw.tile([P, n_blk * W_STRIDE], f32, tag="wb")
            nc.scalar.dma_start(wt[:], wb[:, :])

            for blk in range(n_blk):
                def w(j, blk=blk):
                    return wt[:, blk * W_STRIDE + j:blk * W_STRIDE + j + 1]

                def wdiag(ti, blk=blk):
                    o = (blk * npe + ti) * P
                    return wdt[:, o:o + P]

                for bb in range(b):
                    for h in range(nhalf):
                        base = h * HALF
                        # Runt-free load: 4096B rows + 16B-row tail.
                        X = px.tile([P, hp], f16, tag="x")
                        nc.sync.dma_start(X[:, 0:HALF],
                                          x[blk, bb, :, base:base + HALF])
                        nc.sync.dma_start(X[:, HALF:hp],
                                          x[blk, bb, :,
                                            base + HALF:base + hp])
                        pt = pp.tile([P, HALF], f32, tag="ps")
                        # PE: psum[c] += sum_{j in PE_TAPS} diag(w_j) @ x_j
                        # Tap-major: tap-0 matmuls depend only on the main
                        # load, so the PE starts before the tail arrives.
                        for ti in range(npe):
                            for c in range(HALF // NB):
                                lo = c * NB + PE_TAPS[ti]
                                nc.tensor.matmul(
                                    pt[:, c * NB:(c + 1) * NB],
                                    wdiag(ti),
                                    X[:, lo:lo + NB],
                                    start=(ti == 0),
                                    stop=(ti == npe - 1),
                                    skip_group_check=True,
                                )
                        # ACT: evacuate PSUM -> fp16, fused bias add.
                        s = ps.tile([P, HALF], f16, tag="s")
                        nc.scalar.activation(s[:], pt[:, :], ident,
                                             bias=w(K), scale=1.0)
                        # DVE: remaining even taps + one combine (2x / 4x).
                        y = ps.tile([P, HALF], f16, tag="y")
                        j0 = DVE_TAPS[0]
                        nc.vector.tensor_scalar_mul(
                            y[:], X[:, j0:j0 + HALF], w(j0))
                        for j in DVE_TAPS[1:]:
                            y2 = ps.tile([P, HALF], f16, tag="y")
                            nc.vector.scalar_tensor_tensor(
                                y2[:], X[:, j:j + HALF], w(j),
                                y[:], mult, add)
                            y = y2
                        o = po.tile([P, HALF], f16, tag="o")
                        nc.vector.tensor_tensor(o[:], y[:], s[:], add)
                        # Stores ride the (otherwise idle) gpsimd SWDGE queue.
                        nc.gpsimd.dma_start(out[blk, bb, :, base:base + HALF],
                                            o[:])
    nc.compile()
    return nc


def _prepare(x, w, b):
    x = np.asarray(x, dtype=np.float32)
    w = np.asarray(w, dtype=np.float32)
    b = np.asarray(b, dtype=np.float32)
    npe = len(PE_TAPS)
    # channel-major, left zero-padded time, fp16: [D, B, TPP]
    xp = np.zeros((D, B, TPP), dtype=np.float16)
    xp[:, :, K - 1:K - 1 + T] = x.transpose(2, 0, 1)
    wbt = np.concatenate([w.T, b[:, None]], axis=1).astype(np.float32)  # [D, 5]
    in_maps = []
    for m in range(N_CORES):
        sl = slice(m * D_SH, (m + 1) * D_SH)
        wbm = wbt[sl].reshape(N_BLK, P, W_STRIDE).transpose(1, 0, 2)
        # diag stationary matrices for the PE taps: [P, n_blk*npe*P]
        wdm = np.zeros((P, N_BLK, npe, P), dtype=np.float16)
        rng = np.arange(P)
        for blk in range(N_BLK):
            for ti, tap in enumerate(PE_TAPS):
                wdm[rng, blk, ti, rng] = w[tap, m * D_SH + blk * P + rng]
        in_maps.append({
            "x": np.ascontiguousarray(
                xp[sl].reshape(N_BLK, P, B, TPP).transpose(0, 2, 1, 3)),
            "wd": np.ascontiguousarray(wdm).reshape(P, N_BLK * npe * P),
            "wb": np.ascontiguousarray(wbm).reshape(P, N_BLK * W_STRIDE),
        })
    return in_maps


def _collect(results):
    out = np.empty((B, T, D), dtype=np.float32)
    for m in range(N_CORES):
        o = np.asarray(results[m]["out"]).astype(np.float32)
        o = o.reshape(N_BLK, B, P, T).transpose(1, 3, 0, 2).reshape(B, T, D_SH)
        out[:, :, m * D_SH:(m + 1) * D_SH] = o
    return out


def _run(in_maps, trace=False, **kwargs):
    nc = _build()
    return bass_utils.run_bass_kernel_spmd(
        nc, in_maps, core_ids=list(range(N_CORES)), trace=trace, **kwargs)


def kernel(x, w, b):
    in_maps = _prepare(x, w, b)
    try:
        res = _run(in_maps)
    except Exception:
        # Transient NRT device errors have been observed on a cold first
        # execute; one retry (fresh compile dir) clears them.
        res = _run(in_maps)
    return _collect(res.results)
